# revision 14
# baseline (speedup 1.0000x reference)
"""Soft-DTW-style loss (soft-min of pairwise sq-distances) on Trainium2.

Data-parallel over batch B=8: one batch element per NeuronCore. Per core:
  d[i,j] = ||p_i||^2 + ||t_j||^2 - 2 p_i . t_j            (2048 x 2048)
  S[i]   = sum_j exp(-d[i,j])
computed as one K=9 fp32r matmul producing (2 p.t - t^2) plus an ACT
Exp pass with per-partition bias (-p^2) and fused accum_out row-sum.
Host packs the K-major augmented operands ([pred^T; 1], [2 target^T; -t2])
so the device needs no transposes; the operands are replicated at 4
partition offsets so each j-chunk's matmul runs in its own PE row-group
(LDWEIGHTS overlaps in-flight matmuls only across row-groups).
Host combines: loss = mean(-log(S)).

Self-contained: hardcodes shapes B=8, L=2048, F=8.
"""

import numpy as np
from contextlib import ExitStack

B, L, F = 8, 2048, 8
P = 128          # partition tile height (i rows per tile)
NT = L // P      # 16 i-tiles
KA = F + 1       # augmented contraction dim: 8 features + const row
JC = 512         # j-chunk (one PSUM bank of fp32)
NJ = L // JC     # 4 j-chunks, one PE row-group each

_cache = {}


def _build_nc():
    import concourse.tile as tile
    from concourse import bacc, mybir

    dtf = mybir.dt.float32
    dtr = mybir.dt.float32r
    nc = bacc.Bacc("TRN2", target_bir_lowering=False, debug=False, num_devices=B)
    pa = nc.dram_tensor("pa", [KA, L], dtr, kind="ExternalInput").ap()
    ta = nc.dram_tensor("ta", [KA, L], dtr, kind="ExternalInput").ap()
    pb = nc.dram_tensor("pb", [P, NT], dtf, kind="ExternalInput").ap()
    s_out = nc.dram_tensor("s_out", [P, NT], dtf, kind="ExternalOutput").ap()

    with tile.TileContext(nc) as tc, ExitStack() as ctx:
        sb = ctx.enter_context(tc.tile_pool(name="sb", bufs=1))

        # Operands replicated at partition offsets 0/32/64/96: j-chunk q's
        # matmul contracts in PE row-group q so its LDWEIGHTS overlaps the
        # previous chunk's matmul. targ replica q only needs its own chunk.
        predAT4 = sb.tile([3 * 32 + KA, L], dtr)
        targAT4 = sb.tile([3 * 32 + KA, JC], dtr)
        p2neg = sb.tile([P, NT], dtf)
        S_all = sb.tile([P, NT], dtf)

        # Spread input DMAs over the three parallel DGE queues (sync/scalar
        # HWDGE + gpsimd SWDGE). DMA completion has ~2-4us receipt latency,
        # so land a small "head" (first 4 i-tiles of pred, the bias) first:
        # the first tiles' matmuls/ACT then gate on ~5-20KB transfers while
        # the bulk streams behind them.
        HD = 4 * P  # head: i-columns for tiles 0-3
        # All four 18KB targ chunks gate tile 0, so they go first, spread
        # over the three queues; pred heads follow, then the bulk tails.
        # Per-chunk deps: MM(0,q) needs (pred head q, targ q). Interleave so
        # chunks 0/1 are ready first (PE starts, HAM warms) while chunks 2/3
        # land right behind them.
        nc.scalar.dma_start(p2neg[:], pb)
        nc.gpsimd.dma_start(targAT4[0:KA, :], ta[:, 0:JC])
        nc.sync.dma_start(predAT4[0:KA, :HD], pa[:, :HD])
        nc.gpsimd.dma_start(targAT4[32 : 32 + KA, :], ta[:, JC : 2 * JC])
        nc.scalar.dma_start(predAT4[32 : 32 + KA, :HD], pa[:, :HD])
        nc.sync.dma_start(targAT4[64 : 64 + KA, :], ta[:, 2 * JC : 3 * JC])
        nc.scalar.dma_start(targAT4[96 : 96 + KA, :], ta[:, 3 * JC : 4 * JC])
        nc.sync.dma_start(predAT4[64 : 64 + KA, :HD], pa[:, :HD])
        nc.scalar.dma_start(predAT4[96 : 96 + KA, :HD], pa[:, :HD])
        for q in range(NJ):
            eng = nc.sync if q % 2 == 0 else nc.scalar
            eng.dma_start(predAT4[32 * q : 32 * q + KA, HD:], pa[:, HD:])

        scratch = ctx.enter_context(tc.tile_pool(name="scr", bufs=2))
        with tc.tile_pool(name="pm", bufs=2, space="PSUM") as pm:
            for t in range(NT):
                ptp = pm.tile([P, L], dtf, tag="ptp")  # 4 PSUM banks
                for q in range(NJ):
                    nc.tensor.matmul(
                        ptp[:, q * JC : (q + 1) * JC],
                        predAT4[32 * q : 32 * q + KA, t * P : (t + 1) * P],
                        targAT4[32 * q : 32 * q + KA, :],
                        start=True,
                        stop=True,
                        # explicit: base_partition() auto-derive rejects 96
                        tile_position=(32 * q, 0),
                    )
                eT = scratch.tile([P, L], dtf, tag="eT")
                nc.scalar.activation(
                    eT[:],
                    ptp[:],
                    mybir.ActivationFunctionType.Exp,
                    bias=p2neg[:, t : t + 1],
                    scale=1.0,
                    accum_out=S_all[:, t : t + 1],
                )

        # Ship the first half while tiles 8-15 still run (sync queue); issue
        # the final half from the scalar engine itself so it follows the last
        # ACTIVATE in program order — no cross-engine semaphore hop before
        # the exec-window-closing output DMA.
        nc.sync.dma_start(s_out[:, : NT // 2], S_all[:, : NT // 2])
        nc.scalar.dma_start(s_out[:, NT // 2 :], S_all[:, NT // 2 :])

    nc.compile()
    return nc


def get_nc():
    if "nc" not in _cache:
        _cache["nc"] = _build_nc()
    return _cache["nc"]


def host_prep(pred_b: np.ndarray, target_b: np.ndarray) -> dict:
    """Pack one batch element into the device input layout."""
    pred_b = np.ascontiguousarray(pred_b, dtype=np.float32)
    target_b = np.ascontiguousarray(target_b, dtype=np.float32)

    pa = np.empty((KA, L), np.float32)
    pa[:F] = pred_b.T
    pa[F] = 1.0
    ta = np.empty((KA, L), np.float32)
    ta[:F] = 2.0 * target_b.T
    ta[F] = -np.sum(target_b * target_b, axis=1)
    p2 = np.sum(pred_b * pred_b, axis=1)          # (L,)
    pb = np.ascontiguousarray((-p2).reshape(NT, P).T)  # (128, 16)
    return {
        "pa": np.ascontiguousarray(pa),
        "ta": np.ascontiguousarray(ta),
        "pb": pb,
    }


def reduce_host(s_stack: np.ndarray) -> np.ndarray:
    """(B, 128, 16) row sums -> scalar mean(-log S), fp64 accumulate."""
    loss = -np.log(s_stack.astype(np.float64))
    return np.asarray(loss.mean(), dtype=np.float32)


def run_on_hw(pred: np.ndarray, target: np.ndarray, trace: bool = False):
    from concourse import bass_utils

    nc = get_nc()
    in_maps = [host_prep(pred[b], target[b]) for b in range(B)]
    res = bass_utils.run_bass_kernel_spmd(
        nc, in_maps, core_ids=list(range(B)), trace=trace
    )
    s_stack = np.stack([r["s_out"] for r in res.results])  # (B, 128, 16)
    return reduce_host(s_stack), res


def kernel(pred: np.ndarray, target: np.ndarray) -> np.ndarray:
    pred = np.asarray(pred, dtype=np.float32)
    target = np.asarray(target, dtype=np.float32)
    assert pred.shape == (B, L, F) and target.shape == (B, L, F)
    loss, _ = run_on_hw(pred, target)
    return loss
